# revision 1
# baseline (speedup 1.0000x reference)
"""Trainium2 Bass kernel for nn_MultiHeadAttention_74105365725531.

Multi-head attention with full (n, m)-indexed relative-position key scores
(rpos_k; rpos_v is unused by the reference). Sharding: tensor-parallel over
the 8 heads — one head per NeuronCore. Each core projects Q/K/V for its head,
computes content + relative-position scores, softmax (un-normalized; the
per-row denominators are exported and divided out on the host, which is exact
because the output projection is linear), attention, and its partial output
projection. The host sums the 8 partial output projections.

Self-contained: builds/compiles the Bass program on first call and runs it on
8 NeuronCores via run_bass_kernel_spmd.
"""

import math
import os

import numpy as np
import ml_dtypes

import concourse.bacc as bacc
import concourse.bass as bass
import concourse.mybir as mybir
import concourse.tile as tile
from concourse.bass_utils import run_bass_kernel_spmd

BF16 = mybir.dt.bfloat16
F8 = mybir.dt.float8e4
F32 = mybir.dt.float32
NPBF = ml_dtypes.bfloat16
NPF8 = ml_dtypes.float8_e4m3

BS = 8      # batch
N = 384     # sequence positions
D = 512     # model dim (d_in == d_out)
H = 8       # heads == cores
DK = 64     # head dim
N_CORES = 8
INV_SQRT_DK = 1.0 / math.sqrt(DK)

last_exec_time_ns = None


def build_nc(BS=8, N=384, D=512, DK=64, n_cores=8, stb=None):
    """Build the per-core (SPMD, head-parallel) Bass program."""
    T = BS * N              # tokens
    KC = D // 128           # contraction chunks for projections
    NT = N // 128           # n/m 128-tiles
    PAIRS = N // 2          # rpos position pairs (2 n's packed per matmul)
    BANKS = N // 8          # pos psum banks (4 pairs -> 8 n's per bank)
    STB = stb or (48 if PAIRS % 48 == 0 else PAIRS)  # staging batch (pairs)
    TG = T // 512           # projection token groups

    nc = bacc.Bacc("TRN2", target_bir_lowering=False, debug=False,
                   num_devices=n_cores)

    # ---- I/O ----
    qT = nc.dram_tensor("qT", [D, T], BF16, kind="ExternalInput")
    kT = nc.dram_tensor("kT", [D, T], BF16, kind="ExternalInput")
    vT = nc.dram_tensor("vT", [D, T], BF16, kind="ExternalInput")
    wq = nc.dram_tensor("wq", [D, DK], BF16, kind="ExternalInput")
    wk = nc.dram_tensor("wk", [D, DK], BF16, kind="ExternalInput")
    wv = nc.dram_tensor("wv", [D, DK], BF16, kind="ExternalInput")
    bq = nc.dram_tensor("bq", [DK, 1], F32, kind="ExternalInput")
    bk = nc.dram_tensor("bk", [DK, 1], F32, kind="ExternalInput")
    bv = nc.dram_tensor("bv", [DK, 1], F32, kind="ExternalInput")
    wo = nc.dram_tensor("wo", [DK, D], BF16, kind="ExternalInput")
    rp = nc.dram_tensor("rp", [PAIRS, 128, N], F8, kind="ExternalInput")
    ident = nc.dram_tensor("ident", [128, 128], BF16, kind="ExternalInput")

    wrm = nc.dram_tensor("wrm", [1, 4], F32, kind="ExternalOutput")
    origv = nc.dram_tensor("origv", [DK, T], F32, kind="ExternalOutput")
    outT = nc.dram_tensor("outT", [BS, KC, 128, N], BF16, kind="ExternalOutput")
    sums = nc.dram_tensor("sums", [BS, N], F32, kind="ExternalOutput")

    with tile.TileContext(nc) as tc:
        with (
            tc.tile_pool(name="const", bufs=1) as constp,
            tc.tile_pool(name="persist", bufs=1) as persist,
            tc.tile_pool(name="qin", bufs=9) as qin,
            tc.tile_pool(name="rpin", bufs=4) as rpin,
            tc.tile_pool(name="posb", bufs=3) as posb,
            tc.tile_pool(name="ssb", bufs=4) as ssb,
            tc.tile_pool(name="etp", bufs=6) as etp,
            tc.tile_pool(name="asb", bufs=3) as asbp,
            tc.tile_pool(name="outsb", bufs=2) as outsb,
            tc.tile_pool(name="ps", bufs=2, space="PSUM") as psp,
        ):
            # ---- earliest possible input streaming: q chunks first ----
            q_chunks = []
            for c in range(KC):
                qch = qin.tile([128, T], BF16, name="qchunk", tag="qchunk")
                nc.sync.dma_start(qch[:], qT.ap()[c * 128:(c + 1) * 128, :])
                q_chunks.append(qch)

            # ---- PE warm-up burst (no input deps: memset-fed matmuls) ----
            wseed = constp.tile([128, 512], BF16, name="wseed")
            nc.vector.memset(wseed[:], 0.0)
            wsb = constp.tile([1, 4], F32, name="wsb")
            for wi in range(12):
                wps = psp.tile([128, 512], F32, name="wps", tag="pD")
                nc.tensor.matmul(wps[:], wseed[:, 0:128], wseed[:], start=True, stop=True)
                if wi == 11:
                    nc.vector.tensor_copy(wsb[:], wps[0:1, 0:4])
            nc.sync.dma_start(wrm.ap(), wsb[:])

            # ---- constants / weights in SBUF ----
            identS = constp.tile([128, 128], BF16, name="identS")
            nc.sync.dma_start(identS[:], ident.ap())
            wS = {}
            for nm, w in (("wq", wq), ("wk", wk), ("wv", wv)):
                t = constp.tile([128, KC * DK], BF16, name=nm + "S")
                nc.sync.dma_start(
                    t.rearrange("p (c k) -> p c k", k=DK),
                    w.ap().rearrange("(c p) k -> p c k", p=128),
                )
                wS[nm] = t
            woS = constp.tile([DK, D], BF16, name="woS")
            nc.sync.dma_start(woS[:], wo.ap())
            bS = {}
            for nm, b in (("bq", bq), ("bk", bk), ("bv", bv)):
                t = constp.tile([DK, 1], F32, name=nm + "S")
                nc.sync.dma_start(t[:], b.ap())
                bS[nm] = t

            # ---- persistent activations ----
            QH2 = persist.tile([128, T], BF16, name="QH2")     # qh^T, duplicated rows 64:128
            KH = persist.tile([DK, T], BF16, name="KH")        # kh^T
            VHB = persist.tile([DK, T], BF16, name="VHB")      # vh^T bf16
            OVS = persist.tile([DK, T], F32, name="OVS")       # vh^T f32 (original_v)
            VT = persist.tile([128, BS * NT * (DK + 1)], BF16, name="VT")  # vh [m,d]+ones
            POS_T = persist.tile([128, NT * BANKS * 128], BF16, name="POS_T")
            SUMS_SB = persist.tile([DK + 1, BS * N], F32, name="SUMS_SB")

            # ================= Phase 1: projections =================
            def project(nm, srcT, bias, dst, chunks=None):
                if chunks is None:
                    chunks = []
                    for c in range(KC):
                        ch = qin.tile([128, T], BF16, name="qchunk", tag="qchunk")
                        nc.sync.dma_start(ch[:], srcT.ap()[c * 128:(c + 1) * 128, :])
                        chunks.append(ch)
                for g in range(TG):
                    ps = psp.tile([DK, 512], F32, name="proj_acc", tag="pA")
                    for c in range(KC):
                        nc.tensor.matmul(
                            ps[:],
                            wS[nm][:, c * DK:(c + 1) * DK],
                            chunks[c][:, g * 512:(g + 1) * 512],
                            start=(c == 0), stop=(c == KC - 1),
                        )
                    sl = slice(g * 512, (g + 1) * 512)
                    nc.vector.tensor_scalar_add(dst[0:DK, sl], ps[:], bS[bias][:])

            project("wq", qT, "bq", QH2, chunks=q_chunks)
            # duplicate qh^T into partitions 64:128 (for block-diag staging)
            nc.sync.dma_start(QH2[DK:2 * DK, :], QH2[0:DK, :])

            # ================= Phase 2: relative-position scores =================
            # ST staging: block-diagonal lhsT [128, 32] per pair (2 n's x 8 b,
            # cols 16-31 zero); rhs = streamed rpos pair-chunk [128 (2n x 64d), m].
            ST0 = persist.tile([128, 32 * STB], F8, name="ST0")
            ST1 = persist.tile([128, 32 * STB], F8, name="ST1")
            nc.vector.memset(ST0[:], 0.0)
            nc.vector.memset(ST1[:], 0.0)
            QH2v = QH2.rearrange("p (b pr two) -> p pr two b", two=2, b=BS)

            if True:
                for batch in range(PAIRS // STB):
                    ST = (ST0, ST1)[batch % 2]
                    STv = ST.rearrange("p (g j) -> p g j", j=32)
                    p0 = batch * STB
                    nc.vector.tensor_copy(
                        STv[0:DK, :, 0:8], QH2v[0:DK, p0:p0 + STB, 0, :])
                    nc.vector.tensor_copy(
                        STv[DK:128, :, 8:16], QH2v[DK:128, p0:p0 + STB, 1, :])
                    for g8 in range(STB // 16):
                        rt = rpin.tile([128, 16 * N], F8, name="rtile", tag="rtile")
                        nc.sync.dma_start(
                            rt.rearrange("p (pair m) -> p pair m", pair=16),
                            rp.ap()[p0 + g8 * 16: p0 + g8 * 16 + 16]
                            .rearrange("pair p m -> p pair m"),
                        )
                        for gg in range(16):
                            g = g8 * 16 + gg
                            p = p0 + g
                            qq = p % 4
                            if qq == 0:
                                bank_ps = psp.tile([128, N], F32, name="bank_ps",
                                                   tag="pB")
                            nc.tensor.matmul(
                                bank_ps[32 * qq: 32 * qq + 32, :],
                                ST[:, 32 * g: 32 * g + 32],
                                rt[:, gg * N:(gg + 1) * N],
                                start=True, stop=True,
                                tile_position=(0, 32 * qq),
                            )
                            if qq == 3:
                                bankIdx = p // 4
                                pb = posb.tile([128, N], BF16, name="pb", tag="pb")
                                nc.any.tensor_copy(pb[:], bank_ps[:])
                                ttp = psp.tile([128, N], BF16, name="ttp", tag="pC")
                                for c in range(NT):
                                    nc.tensor.transpose(
                                        ttp[:, c * 128:(c + 1) * 128],
                                        pb[:, c * 128:(c + 1) * 128], identS[:])
                                dst = POS_T.rearrange(
                                    "p (c B l) -> p c B l", B=BANKS, l=128)
                                nc.any.tensor_copy(
                                    dst[:, :, bankIdx, :],
                                    ttp.rearrange("p (c l) -> p c l", l=128))
            project("wk", kT, "bk", KH)
            project("wv", vT, "bv", OVS)
            nc.scalar.dma_start(origv.ap(), OVS[:])
            nc.vector.tensor_copy(VHB[:], OVS[:])

            # vh [m, d] transposes (+ ones column for softmax denominators)
            nc.vector.memset(
                VT.rearrange("p (x u) -> p x u", u=DK + 1)[:, :, DK:DK + 1], 1.0)
            for b in range(BS):
                for c in range(NT):
                    tp = psp.tile([128, DK], BF16, name="tp", tag="pC")
                    nc.tensor.transpose(
                        tp[:], VHB[0:DK, b * N + c * 128: b * N + (c + 1) * 128],
                        identS[0:DK, 0:DK])
                    off = (b * NT + c) * (DK + 1)
                    nc.vector.tensor_copy(VT[:, off:off + DK], tp[:])

            # ================= Phase 3: scores + softmax + attention =================
            POS_Tv = POS_T.rearrange("p (c B q z i j) -> p c B q z i j",
                                     B=BANKS, q=4, z=2, i=2, j=8)
            if True:
                for b in range(BS):
                    a_ps = psp.tile([DK + 1, N], F32, name="a_ps", tag="pD")
                    for c in range(NT):
                        s_ps = psp.tile([128, N], F32, name="s_ps", tag="pA")
                        nc.tensor.matmul(
                            s_ps[:],
                            KH[0:DK, b * N + c * 128: b * N + (c + 1) * 128],
                            QH2[0:DK, b * N:(b + 1) * N],
                            start=True, stop=True,
                        )
                        s_sb = ssb.tile([128, N], F32, name="s_sb", tag="ssb")
                        nc.vector.tensor_add(
                            s_sb.rearrange("p (B q i) -> p B q i", q=4, i=2),
                            s_ps.rearrange("p (B q i) -> p B q i", q=4, i=2),
                            POS_Tv[:, c, :, :, 0, :, b])
                        et = etp.tile([128, N], BF16, name="et", tag="et")
                        nc.scalar.activation(
                            et[:], s_sb[:], mybir.ActivationFunctionType.Exp,
                            scale=INV_SQRT_DK)
                        off = (b * NT + c) * (DK + 1)
                        nc.tensor.matmul(
                            a_ps[:], VT[:, off:off + DK + 1], et[:],
                            start=(c == 0), stop=(c == NT - 1),
                        )
                    a_sb = asbp.tile([DK, N], BF16, name="a_sb", tag="asb")
                    nc.vector.tensor_copy(a_sb[:], a_ps[0:DK, :])
                    nc.vector.tensor_copy(
                        SUMS_SB[DK:DK + 1, b * N:(b + 1) * N], a_ps[DK:DK + 1, :])
                    out_sb = outsb.tile([128, KC * N], BF16, name="out_sb", tag="outsb")
                    for c2 in range(KC):
                        o_ps = psp.tile([128, N], F32, name="o_ps", tag="pB")
                        nc.tensor.matmul(
                            o_ps[:], woS[:, c2 * 128:(c2 + 1) * 128], a_sb[:],
                            start=True, stop=True,
                        )
                        nc.any.tensor_copy(
                            out_sb[:, c2 * N:(c2 + 1) * N], o_ps[:])
                    nc.scalar.dma_start(
                        outT.ap()[b].rearrange("c p m -> p c m"),
                        out_sb.rearrange("p (c m) -> p c m", m=N))
                nc.sync.dma_start(
                    sums.ap().rearrange("b m -> (b m)")[None, :],
                    SUMS_SB[DK:DK + 1, :])

    nc.compile()
    return nc


_NC = None


def _get_nc():
    global _NC
    if _NC is None:
        _NC = build_nc(BS, N, D, DK, N_CORES)
    return _NC


def prep_inputs(q, k, v, rpos_k, Wq, bq, Wk, bk, Wv, bv, Wo, bo):
    """Host-side sharding/layout prep. Returns in_maps for the 8 cores."""
    T = BS * N
    qT = np.ascontiguousarray(q.reshape(T, D).astype(NPBF).T)
    kT = np.ascontiguousarray(k.reshape(T, D).astype(NPBF).T)
    vT = np.ascontiguousarray(v.reshape(T, D).astype(NPBF).T)
    identity = np.eye(128, dtype=NPBF)
    in_maps = []
    for h in range(H):
        sl = slice(h * DK, (h + 1) * DK)
        A = rpos_k[:, :, h, :].astype(NPF8)          # [n, m, d]
        A2 = A.reshape(N // 2, 2, N, DK)             # [pair, i, m, d]
        rp_h = np.ascontiguousarray(
            A2.transpose(0, 1, 3, 2)                 # [pair, i, d, m]
        ).reshape(N // 2, 128, N)
        in_maps.append({
            "qT": qT, "kT": kT, "vT": vT,
            "wq": np.ascontiguousarray(Wq[:, sl].astype(NPBF)),
            "wk": np.ascontiguousarray(Wk[:, sl].astype(NPBF)),
            "wv": np.ascontiguousarray(Wv[:, sl].astype(NPBF)),
            "bq": np.ascontiguousarray(bq[sl].astype(np.float32).reshape(DK, 1)),
            "bk": np.ascontiguousarray(bk[sl].astype(np.float32).reshape(DK, 1)),
            "bv": np.ascontiguousarray(bv[sl].astype(np.float32).reshape(DK, 1)),
            "wo": np.ascontiguousarray(
                (Wo[sl, :] * INV_SQRT_DK).astype(NPBF)),
            "rp": rp_h,
            "ident": identity,
        })
    return in_maps


def _maybe_install_trace_shim():
    """Install antenv.axon_hooks (NTFF profiling) when tracing is requested."""
    import sys
    import types
    import ctypes
    import contextlib

    if "antenv.axon_hooks" in sys.modules:
        return
    so_path = "/opt/axon/libaxon_pjrt.so"
    lib = ctypes.CDLL(so_path)
    if not hasattr(lib, "axon_start_nrt_profile"):
        return
    lib.axon_start_nrt_profile.argtypes = [ctypes.POINTER(ctypes.c_int64),
                                           ctypes.c_size_t]
    lib.axon_start_nrt_profile.restype = ctypes.c_int64
    lib.axon_stop_nrt_profile.argtypes = [ctypes.c_char_p]
    lib.axon_stop_nrt_profile.restype = ctypes.c_int64

    @contextlib.contextmanager
    def _hook(output_dir, device_ids):
        import jax
        jax.devices()
        if device_ids:
            ids = (ctypes.c_int64 * len(device_ids))(*device_ids)
            rc = lib.axon_start_nrt_profile(ids, len(device_ids))
        else:
            rc = lib.axon_start_nrt_profile(None, 0)
        if rc != 0:
            raise RuntimeError(f"axon_start_nrt_profile rc={rc}")
        try:
            yield
        finally:
            n = lib.axon_stop_nrt_profile(str(output_dir).encode())
            print(f"profile: {n} file(s) in {output_dir}")

    mod = types.ModuleType("antenv.axon_hooks")
    mod.get_axon_ntff_profile_hook = lambda: _hook
    mod.set_axon_ntff_profile_hook = lambda h: None
    sys.modules["antenv.axon_hooks"] = mod


def kernel(**inputs):
    global last_exec_time_ns
    q = np.asarray(inputs["q"], np.float32)
    k = np.asarray(inputs["k"], np.float32)
    v = np.asarray(inputs["v"], np.float32)
    rpos_k = np.asarray(inputs["rpos_k"], np.float32)
    Wq = np.asarray(inputs["Wq"], np.float32)
    bq = np.asarray(inputs["bq"], np.float32)
    Wk = np.asarray(inputs["Wk"], np.float32)
    bk = np.asarray(inputs["bk"], np.float32)
    Wv = np.asarray(inputs["Wv"], np.float32)
    bv = np.asarray(inputs["bv"], np.float32)
    Wo = np.asarray(inputs["Wo"], np.float32)
    bo = np.asarray(inputs["bo"], np.float32)

    trace = bool(os.environ.get("KERNEL_TRACE"))
    if trace:
        _maybe_install_trace_shim()

    nc = _get_nc()
    in_maps = prep_inputs(q, k, v, rpos_k, Wq, bq, Wk, bk, Wv, bv, Wo, bo)
    res = run_bass_kernel_spmd(nc, in_maps, core_ids=list(range(N_CORES)),
                               trace=trace)
    last_exec_time_ns = res.exec_time_ns

    original_v = np.empty((BS, N, D), np.float32)
    output = np.zeros((BS, N, D), np.float32)
    for h in range(H):
        r = res.results[h]
        original_v[:, :, h * DK:(h + 1) * DK] = (
            r["origv"].T.reshape(BS, N, DK))
        # outT [BS, KC, 128, N] -> [BS, N, D]; divide by softmax denominators
        out_h = r["outT"].astype(np.float32).reshape(BS, D, N).transpose(0, 2, 1)
        output += out_h / r["sums"][:, :, None]
    output += bo
    return original_v, output

